# revision 25
# baseline (speedup 1.0000x reference)
"""Trainium2 Bass kernel for nn_MultiHeadAttention_30846455119878.

8-core strategy:
  - Attention phase is head-sharded: core m owns heads {2m, 2m+1}. Every core
    computes q/k/v projections for its 2 heads over all B*T tokens, then causal
    softmax attention per (batch, head).
  - The output projection contracts over ALL heads, so instead of an expensive
    AllReduce of [B,T,C] partials, each batch's attention output features
    ([128 feats x T]) are exchanged with a small AllToAll that re-shards from
    heads -> tokens. Each core then computes the full output projection for its
    1/8 token slice (contraction over all 1024 features) plus bias, locally.
  - Host side: x is passed pre-transposed as x^T [C, B*T] in bf16 (fp32 can't
    DMA-transpose on TRN2); all matmuls run bf16 x bf16 -> fp32 PSUM.

Layouts (per core):
  xt   [128, CK, B*T] bf16   xt[p, o, t] = x[t, o*128+p]
  wq/wk/wv [128, CK, 128] bf16   w[p, o, f] = W[o*128+p, head_lo*64... ] (2 heads stacked on f)
  wo   [128, 8, C] bf16     wo[p, j, c] = Wo[j*128+p, c]
  bo   [1, C] bf16
  mask [128, 4, 512] bf16   mask[p, j, t] = 1 if t >= p + j*128 else 0  (causal diag blocks)
  out  [B, T/8, C] fp32     core m holds tokens [m*T/8, (m+1)*T/8) of every batch
"""

import sys

if "/opt/trn_rl_repo" not in sys.path:
    sys.path.insert(0, "/opt/trn_rl_repo")

import numpy as np
import ml_dtypes

import concourse.bass as bass
import concourse.tile as tile
from concourse import bacc, mybir
from concourse.bass_utils import run_bass_kernel_spmd
from concourse.tile_rust import add_dep_helper

BF16 = ml_dtypes.bfloat16

# Full problem dims
B_FULL, T_FULL, C_FULL, H_FULL, D_HEAD = 4, 2048, 1024, 16, 64
N_CORES = 8
HPC = H_FULL // N_CORES  # heads per core = 2
F = HPC * D_HEAD         # per-core attention feature rows = 128
TCH = 512                # query-chunk (free dim of score matmuls)
D = D_HEAD


def build_nc(B=B_FULL, T=T_FULL, C=C_FULL, debug=False):
    """Build the SPMD Bass graph (same graph on all 8 cores)."""
    dt = mybir.dt
    CK = C // 128        # contraction chunks for projections
    NTC = T // TCH       # query chunks per sequence
    NSB = T // 128       # key blocks per sequence
    SBB = TCH // 128     # key blocks that overlap one query chunk diagonal = 4
    TS = (B * T) // (B * N_CORES)  # token shard per (batch, core) = T // 8
    CO = H_FULL * D_HEAD  # output feature dim (Wo cols) = 1024
    TT = 128 if TS % 128 == 0 else TS  # token tile for output projection
    scale = float(1.0 / np.sqrt(C))

    nc = bacc.Bacc()
    xt_d = nc.declare_dram_parameter("xt", [128, CK, B * T], dt.bfloat16, isOutput=False)
    wq_d = nc.declare_dram_parameter("wq", [128, CK, F], dt.bfloat16, isOutput=False)
    wk_d = nc.declare_dram_parameter("wk", [128, CK, F], dt.bfloat16, isOutput=False)
    wv_d = nc.declare_dram_parameter("wv", [128, CK, F], dt.bfloat16, isOutput=False)
    wo_d = nc.declare_dram_parameter("wo", [128, N_CORES, CO], dt.bfloat16, isOutput=False)
    bo_d = nc.declare_dram_parameter("bo", [1, CO], dt.bfloat16, isOutput=False)
    mask_d = nc.declare_dram_parameter("mask", [128, SBB, TCH], dt.bfloat16, isOutput=False)
    out_d = nc.declare_dram_parameter("out", [B, TS, CO], dt.float32, isOutput=True)

    dbg = {}
    if debug:
        dbg["attn"] = nc.declare_dram_parameter("dbg_attn", [D, T], dt.bfloat16, isOutput=True)
        dbg["v1"] = nc.declare_dram_parameter("dbg_v1", [128, T // 128, HPC, 80], dt.bfloat16, isOutput=True)
        dbg["rcv"] = nc.declare_dram_parameter("dbg_rcv", [128, N_CORES, TS], dt.bfloat16, isOutput=True)
    NH = 2 if NTC % 2 == 0 else 1   # token halves per batch for split AllToAll
    TPH = T // NH                    # tokens per half
    TSH = TPH // N_CORES             # tokens per (half, core) shard
    SLOTS_H = (NTC // NH) * HPC      # (head, chunk) slots per half
    cc_in = [nc.dram_tensor(f"cc_in{b}_{hf}", [N_CORES, F, TSH], dt.bfloat16)
             for b in range(B) for hf in range(NH)]
    cc_out = [nc.dram_tensor(f"cc_out{b}_{hf}", [N_CORES, F, TSH], dt.bfloat16)
              for b in range(B) for hf in range(NH)]
    rg = [list(range(N_CORES))]

    with tile.TileContext(nc) as tc:
        from contextlib import ExitStack

        with ExitStack() as ctx:
            wpool = ctx.enter_context(tc.tile_pool(name="w", bufs=1))
            xpool = ctx.enter_context(tc.tile_pool(name="xt", bufs=3))
            qkpool = ctx.enter_context(tc.tile_pool(name="qk", bufs=2))
            v1pool = ctx.enter_context(tc.tile_pool(name="v1", bufs=2))
            epool = ctx.enter_context(tc.tile_pool(name="exp", bufs=6))
            apool = ctx.enter_context(tc.tile_pool(name="attn", bufs=4))
            recpool = ctx.enter_context(tc.tile_pool(name="rec", bufs=3))
            aupool = ctx.enter_context(tc.tile_pool(name="attu", bufs=2))
            denpool = ctx.enter_context(tc.tile_pool(name="den", bufs=2))
            rcvpool = ctx.enter_context(tc.tile_pool(name="rcv", bufs=2))
            outpool = ctx.enter_context(tc.tile_pool(name="osb", bufs=2))
            psA = ctx.enter_context(tc.tile_pool(name="psA", bufs=5, space="PSUM"))
            psB = ctx.enter_context(tc.tile_pool(name="psB", bufs=3, space="PSUM"))

            # resident constants
            wq_sb = wpool.tile([128, CK, F], dt.bfloat16, tag="wq")
            wk_sb = wpool.tile([128, CK, F], dt.bfloat16, tag="wk")
            wv_sb = wpool.tile([128, CK, F], dt.bfloat16, tag="wv")
            wo_sb = wpool.tile([128, N_CORES, CO], dt.bfloat16, tag="wo")
            bo_sb = wpool.tile([1, CO], dt.bfloat16, tag="bo")
            mask_sb = wpool.tile([128, SBB, TCH], dt.bfloat16, tag="mask")
            ones_sb = wpool.tile([D + 1, 128], dt.bfloat16, tag="ones")
            nc.sync.dma_start(out=wq_sb, in_=wq_d[:, :, :])
            nc.sync.dma_start(out=wk_sb, in_=wk_d[:, :, :])
            nc.sync.dma_start(out=wv_sb, in_=wv_d[:, :, :])
            nc.sync.dma_start(out=wo_sb, in_=wo_d[:, :, :])
            nc.sync.dma_start(out=bo_sb, in_=bo_d[:, :])
            nc.sync.dma_start(out=mask_sb, in_=mask_d[:, :, :])
            nc.vector.memset(ones_sb, 1.0)

            cc_insts = []
            for b in range(B):
                # ---- phase A: q/k projections ([d, t] layout) and v ([s, d] layout)
                qT = qkpool.tile([F, T], dt.bfloat16, tag="qT")
                kT = qkpool.tile([F, T], dt.bfloat16, tag="kT")
                v1 = v1pool.tile([128, NSB, HPC, 80], dt.bfloat16, tag="v1")
                nc.vector.memset(v1[:, :, :, D:D + 1], 1.0)
                for tcb in range(NTC):
                    g0 = b * T + tcb * TCH
                    xt_sb = xpool.tile([128, CK, TCH], dt.bfloat16, tag="xt")
                    nc.sync.dma_start(out=xt_sb, in_=xt_d[:, :, g0:g0 + TCH])
                    for w_sb, dstT in ((wq_sb, qT), (wk_sb, kT)):
                        ps = psA.tile([128, TCH], dt.float32, tag="mm")
                        for o in range(CK):
                            nc.tensor.matmul(
                                ps, lhsT=w_sb[:, o, :], rhs=xt_sb[:, o, :],
                                start=(o == 0), stop=(o == CK - 1),
                            )
                        nc.vector.tensor_copy(
                            out=dstT[:, tcb * TCH:(tcb + 1) * TCH], in_=ps
                        )
                    # v directly in [s, d] layout: v[s, f] = sum_c x[s, c] Wv[c, f]
                    for ssub in range(SBB):
                        vps_full = psA.tile([128, TCH], dt.float32, tag="mm", name=f"vps_{b}_{tcb}_{ssub}")
                        vps = vps_full[:, 0:F]
                        for o in range(CK):
                            nc.tensor.matmul(
                                vps,
                                lhsT=xt_sb[:, o, ssub * 128:(ssub + 1) * 128],
                                rhs=wv_sb[:, o, :],
                                start=(o == 0), stop=(o == CK - 1),
                            )
                        st = tcb * SBB + ssub
                        for h in range(HPC):
                            nc.vector.tensor_copy(
                                out=v1[:, st, h, 0:D], in_=vps[:, h * D:(h + 1) * D]
                            )
                if debug and b == 0:
                    nc.scalar.dma_start(out=dbg["v1"][:, :, :, :], in_=v1)

                # ---- phase B: causal attention, both heads interleaved
                SLOTS = NTC * HPC
                attn_h = [apool.tile([D, T], dt.bfloat16, tag="attn", name=f"attn_{b}_{hh}") for hh in range(HPC)]
                att_un = aupool.tile([D, SLOTS, TCH], dt.bfloat16, tag="attu")
                den_b = [denpool.tile([D + 1, SLOTS_H * TCH], dt.float32, tag="den", name=f"den_{b}_{hf}") for hf in range(NH)]
                for tcb in range(NTC):
                    hf = tcb // (NTC // NH)
                    att_ps = [psB.tile([D + 1, TCH], dt.float32, tag="att", name=f"attps_{b}_{tcb}_{hh}") for hh in range(HPC)]
                    nsb = SBB * (tcb + 1)
                    for sb in range(nsb):
                        j0 = sb - SBB * tcb
                        # columns t < j0*128 of this (key-block, query-chunk) pair are
                        # fully causal-masked -> skip them in scores/exp/mask/att
                        c0 = j0 * 128 if j0 > 0 else 0
                        ets = []
                        for h in range(HPC):
                            s_ps = psA.tile([128, TCH], dt.float32, tag="mm")
                            nc.tensor.matmul(
                                s_ps[:, c0:TCH],
                                lhsT=kT[h * D:(h + 1) * D, sb * 128:(sb + 1) * 128],
                                rhs=qT[h * D:(h + 1) * D, tcb * TCH + c0:(tcb + 1) * TCH],
                                start=True, stop=True,
                                tile_position=(h * D, 0),
                            )
                            et = epool.tile([128, TCH], dt.bfloat16, tag="exp")
                            nc.scalar.activation(
                                out=et[:, c0:TCH], in_=s_ps[:, c0:TCH],
                                func=mybir.ActivationFunctionType.Exp, scale=scale,
                            )
                            if j0 >= 0:
                                nc.vector.tensor_mul(
                                    et[:, c0:TCH], et[:, c0:TCH],
                                    mask_sb[:, j0, c0:TCH],
                                )
                            ets.append(et)
                        for h in range(HPC):
                            nc.tensor.matmul(
                                att_ps[h][:, c0:TCH],
                                lhsT=v1[:, sb, h, 0:D + 1], rhs=ets[h][:, c0:TCH],
                                start=(sb == 0), stop=(sb == nsb - 1),
                            )
                    for h in range(HPC):
                        slot = tcb * HPC + h
                        sloth = slot - hf * SLOTS_H
                        # copy unnormalized attention + denominator out of PSUM
                        nc.vector.tensor_copy(out=att_un[:, slot, :], in_=att_ps[h][0:D, :])
                        nc.vector.tensor_copy(
                            out=den_b[hf][D:D + 1, sloth * TCH:(sloth + 1) * TCH],
                            in_=att_ps[h][D:D + 1, :],
                        )
                    if (tcb + 1) % (NTC // NH) == 0:
                        # this token-half is complete: batch-reciprocal its
                        # denominators across 128 lanes, normalize, stage, exchange
                        den_t = recpool.tile([128, SLOTS_H * TCH // 128], dt.float32, tag="dent")
                        nc.sync.dma_start(out=den_t, in_=den_b[hf][D:D + 1, :])
                        rec_t = recpool.tile([128, SLOTS_H * TCH // 128], dt.bfloat16, tag="rect")
                        with nc.allow_low_precision(reason="bf16 softmax denom recip is plenty at rel-err 2e-2"):
                            nc.vector.reciprocal(out=rec_t, in_=den_t)
                        rec_all = recpool.tile([1, SLOTS_H * TCH], dt.bfloat16, tag="recall")
                        nc.sync.dma_start(out=rec_all, in_=rec_t)
                        stg_insts = []
                        for tcb2 in range(hf * (NTC // NH), tcb + 1):
                            for h in range(HPC):
                                slot = tcb2 * HPC + h
                                sloth = slot - hf * SLOTS_H
                                rb_ps = psA.tile([D, TCH], dt.float32, tag="mm")
                                nc.tensor.matmul(
                                    rb_ps, lhsT=ones_sb[0:1, 0:D],
                                    rhs=rec_all[0:1, sloth * TCH:(sloth + 1) * TCH],
                                    start=True, stop=True,
                                )
                                rb_sb = recpool.tile([D, TCH], dt.bfloat16, tag="recbc")
                                nc.scalar.activation(
                                    out=rb_sb, in_=rb_ps,
                                    func=mybir.ActivationFunctionType.Copy,
                                )
                                nc.vector.tensor_mul(
                                    attn_h[h][:, tcb2 * TCH:(tcb2 + 1) * TCH],
                                    att_un[:, slot, :], rb_sb,
                                )
                        for h in range(HPC):
                            for j in range(N_CORES):
                                stg_insts.append(nc.scalar.dma_start(
                                    out=cc_in[b * NH + hf][j, h * D:(h + 1) * D, :],
                                    in_=attn_h[h][:, hf * TPH + j * TSH: hf * TPH + (j + 1) * TSH],
                                ).ins)
                        cc = nc.gpsimd.collective_compute(
                            "AllToAll", mybir.AluOpType.bypass, replica_groups=rg,
                            ins=[cc_in[b * NH + hf].ap().opt()],
                            outs=[cc_out[b * NH + hf].ap().opt()],
                        )
                        for s in stg_insts:
                            add_dep_helper(cc.ins, s, sync=True, reason="cc_in RAW")
                        cc_insts.append(cc.ins)
                if debug and b == 0:
                    nc.scalar.dma_start(out=dbg["attn"][:, :], in_=attn_h[0])

            # ---- phase C: output projection on this core's token shards
            for b in range(B):
                rcv = rcvpool.tile([128, NH, N_CORES, TSH], dt.bfloat16, tag="rcv")
                for hf in range(NH):
                    rcv_rd = nc.scalar.dma_start(
                        out=rcv[:, hf, :, :],
                        in_=cc_out[b * NH + hf][:, :, :].rearrange("j p t -> p j t"),
                    )
                    add_dep_helper(rcv_rd.ins, cc_insts[b * NH + hf], sync=True, reason="cc_out RAW")
                if debug and b == 0:
                    nc.scalar.dma_start(out=dbg["rcv"][:, :, :], in_=rcv[:, 0, :, :])
                for hf in range(NH):
                    for tt in range(TSH // TT if TSH >= TT else 1):
                        tw = min(TT, TSH)
                        for c2 in range(CO // 512):
                            ps = psA.tile([128, TCH], dt.float32, tag="mm")
                            for j in range(N_CORES):
                                nc.tensor.matmul(
                                    ps[0:tw, 0:512],
                                    lhsT=rcv[:, hf, j, tt * tw:(tt + 1) * tw],
                                    rhs=wo_sb[:, j, c2 * 512:(c2 + 1) * 512],
                                    start=(j == 0), stop=False,
                                )
                            nc.tensor.matmul(
                                ps[0:tw, 0:512],
                                lhsT=ones_sb[0:1, 0:tw],
                                rhs=bo_sb[0:1, c2 * 512:(c2 + 1) * 512],
                                start=False, stop=True,
                            )
                            osb = outpool.tile([TT, 512], dt.float32, tag="osb")
                            nc.vector.tensor_copy(out=osb[0:tw, :], in_=ps[0:tw, 0:512])
                            nc.scalar.dma_start(
                                out=out_d[b, hf * TSH + tt * tw: hf * TSH + (tt + 1) * tw,
                                          c2 * 512:(c2 + 1) * 512],
                                in_=osb[0:tw, :],
                            )

    nc.finalize()
    return nc


def prep_inputs(x, Wq, Wk, Wv, Wo, bo):
    """Host-side shard/layout prep. Returns in_maps for the 8 cores."""
    B, T, C = x.shape
    H = Wq.shape[0]
    CK = C // 128
    SBB = TCH // 128

    x = np.asarray(x, dtype=np.float32)
    xt = np.ascontiguousarray(x.reshape(B * T, C).T.astype(BF16))  # [C, B*T]
    xt = np.ascontiguousarray(xt.reshape(CK, 128, B * T).transpose(1, 0, 2))

    CO = Wo.shape[1]
    wo_h = np.ascontiguousarray(
        np.asarray(Wo, np.float32).astype(BF16).reshape(N_CORES, 128, CO).transpose(1, 0, 2)
    )
    bo_h = np.asarray(bo, np.float32).astype(BF16).reshape(1, CO)

    p = np.arange(128)[:, None, None]
    j = np.arange(SBB)[None, :, None]
    t = np.arange(TCH)[None, None, :]
    mask_h = (t >= p + j * 128).astype(BF16)

    in_maps = []
    for m in range(N_CORES):
        maps = {"xt": xt, "wo": wo_h, "bo": bo_h, "mask": mask_h}
        for name, W in (("wq", Wq), ("wk", Wk), ("wv", Wv)):
            Ws = np.concatenate(
                [np.asarray(W[HPC * m + i], np.float32) for i in range(HPC)], axis=1
            )  # [C, F]
            maps[name] = np.ascontiguousarray(
                Ws.astype(BF16).reshape(CK, 128, F).transpose(1, 0, 2)
            )
        in_maps.append(maps)
    return in_maps


_NC_CACHE = {}


def _get_nc(B, T, C):
    key = (B, T, C)
    if key not in _NC_CACHE:
        _NC_CACHE[key] = build_nc(B, T, C)
    return _NC_CACHE[key]


def kernel(x, Wq, Wk, Wv, Wo, bo, _trace=False):
    x = np.asarray(x)
    B, T, C = x.shape
    nc = _get_nc(B, T, C)
    in_maps = prep_inputs(x, Wq, Wk, Wv, Wo, bo)
    res = run_bass_kernel_spmd(
        nc, in_maps, core_ids=list(range(N_CORES)), trace=_trace
    )
    NTC = T // TCH
    NH = 2 if NTC % 2 == 0 else 1
    TPH = T // NH
    TSH = TPH // N_CORES
    CO = np.asarray(Wo).shape[1]
    out = np.empty((B, T, CO), dtype=np.float32)
    for m in range(N_CORES):
        r = res.results[m]["out"]
        for hf in range(NH):
            out[:, hf * TPH + m * TSH: hf * TPH + (m + 1) * TSH, :] = \
                r[:, hf * TSH:(hf + 1) * TSH, :]
    if _trace:
        kernel.last_result = res
    return out
